# revision 32
# baseline (speedup 1.0000x reference)
"""Trainium2 Bass kernel for nn_AttnModel (BiAttn x3 + tiny FC + batch-softmax tile).

Contract: kernel(**inputs) takes the FULL inputs (a_emb/v_emb/l_emb [32,1024,32],
fc1_w [64,64], fc1_b [64], fc2_w [1,64]) and returns the FULL output [32,1024,64].

Strategy (8 NeuronCores, data-parallel over batch, 4 batches/core = 12
(batch, pair) "units"/core; pairs (a,v),(a,l),(v,l)). The reference only uses
row 0 of each BiAttn output, so per unit we need row/col softmax reductions of
E = exp(f @ g^T) plus row 0 / col 0:
    S chunk [128,1024] = f32r matmuls (K=32, 1 cyc/row)          (PE)
    E chunk = exp(S) -> bf16 SBUF, FD=1024 activations           (ScalarE, the
        bottleneck: 12 x 1M exps/core ~ 100us busy, fully saturated)
    rowsums r_i: bf16 4x tensor_scalar pass with accum_out       (DVE)
    colsums c_j: indicator-matmuls accumulate all 4 units of a
        pack into one [8,512] psum bank, trailing the exps by
        one chunk so the PE never stalls                         (PE)
    w2 = E[:,0]/r (strided E view), o2 = sum_i w2_i f_i          (DVE+PE)
    w1 = E[0,:]/c via batched 4-row transposes + one reciprocal,
        o1 = sum_j w1_j g_j; Bi row = [o1*f_0, o2*g_0]           (PE+DVE)
  Tiny FC (h kept transposed so fc1_b is a per-partition bias) -> logits;
  exp(Ci) AllGathered twice (units 0-5 early/hidden, 6-11 at the end; the
  batch-softmax denominators need all 32 batches); alpha-weighted rows are
  broadcast-written via a single stride-0 DMA as bf16.
  E is triple-buffered so unit u+1's exps never wait on unit u-2's readers.
"""
import numpy as np
import ml_dtypes

import concourse.bass as bass
import concourse.bacc as bacc
import concourse.tile as tile
import concourse.mybir as mybir
from concourse.bass_utils import run_bass_kernel_spmd
from concourse.tile_rust import add_dep_helper

F32 = mybir.dt.float32
F32R = mybir.dt.float32r
BF16 = mybir.dt.bfloat16
I16 = mybir.dt.int16
AF = mybir.ActivationFunctionType

B, U, D = 32, 1024, 32
NCORES = 8
BPC = B // NCORES          # batches per core = 4
NU = 3 * BPC               # units per core = 12
NPACK = NU // 4            # packs of 4 units = 3
NCH = U // 128             # i-chunks = 8
PAIRS = [(0, 1), (0, 2), (1, 2)]  # (f,g) emb indices for pair k; 0=a 1=v 2=l

# Schraudolph fast-exp constants (bf16 bit-pattern domain): for chunks
# offloaded to the DVE, exp(x) ~= bitcast_bf16(int16(x*SCHR_A + SCHR_B)).
# Max rel err ~4%, zero-mean; the constant bias cancels in softmax ratios.
SCHR_A = 128.0 / np.log(2.0)
SCHR_B = 16256.0 - 7.42
# Chunks computed on DVE per unit, staggered by unit parity so EVERY
# chunk-step pairs exactly one ACT chunk with one DVE chunk across the two
# units of a pair (keeps both engines busy while the PE streams S/colsums).
# Even units keep chunk 0 on ACT (exact row-0 numerators); odd units'
# row-0 numerators are Schraudolph (~2% noise, well within budget).
DVE_CHUNKS_BY_PARITY = ((1, 3, 5, 7), (0, 2, 4, 6))

_DEBUG = False
import os as _os
S_F32R = _os.environ.get("S_F32R", "0") == "1"


def build_program(repeat=1):
    nc = bacc.Bacc("TRN2", target_bir_lowering=False, debug=False, num_devices=NCORES)

    eblob = nc.dram_tensor("eblob", [NPACK, 128, 2048], F32R, kind="ExternalInput")
    nblob = nc.dram_tensor("nblob", [NPACK, 128, 2048], BF16, kind="ExternalInput")
    fc1T = nc.dram_tensor("fc1T", [64, 64], F32, kind="ExternalInput")
    fc1b = nc.dram_tensor("fc1b", [64, 1], F32, kind="ExternalInput")
    fc2T = nc.dram_tensor("fc2T", [64, 1], F32, kind="ExternalInput")
    out = nc.dram_tensor("out", [BPC, U, 2 * D], BF16, kind="ExternalOutput")
    if _DEBUG:
        dbg_bi = nc.dram_tensor("dbg_bi", [NU, 64], F32, kind="ExternalOutput")
        dbg_eci = nc.dram_tensor("dbg_eci", [NU, 1], F32, kind="ExternalOutput")
        dbg_rows = nc.dram_tensor("dbg_rows", [BPC, 64], F32, kind="ExternalOutput")

    ident_np = np.eye(128, dtype=np.float32)
    ecols_np = np.zeros((128, 64), np.float32)
    for t4 in range(4):
        for h4 in range(2):
            ecols_np[:, 8 * (2 * t4 + h4) + 4 * h4 + t4] = 1.0
    sel4_np = np.zeros((128, 4), np.float32)
    for t4 in range(4):
        sel4_np[32 * t4, t4] = 1.0
    sel_np = np.zeros((NU, BPC), np.float32)
    for r in range(NU):
        sel_np[r, r // 3] = 1.0

    with tile.TileContext(nc) as tc:
        from contextlib import ExitStack
        ctx = ExitStack()
        consts = ctx.enter_context(tc.tile_pool(name="consts", bufs=1))
        bigp = ctx.enter_context(tc.tile_pool(name="big", bufs=1))
        epool = ctx.enter_context(tc.tile_pool(name="epool", bufs=1))
        packp = ctx.enter_context(tc.tile_pool(name="packp", bufs=3))
        tailp = ctx.enter_context(tc.tile_pool(name="tailp", bufs=1))
        dramp = ctx.enter_context(tc.tile_pool(name="dramp", bufs=1, space="DRAM"))

        sps = ctx.enter_context(tc.tile_pool(name="sps", bufs=1, space="PSUM"))
        csps = ctx.enter_context(tc.tile_pool(name="csps", bufs=1, space="PSUM"))
        miscp = ctx.enter_context(tc.tile_pool(name="miscp", bufs=1, space="PSUM"))
        # one shared PSUM bank for all small matmul outputs, sliced by column
        # range. Per-pack regions (tpe/tpc/o) alternate by pack parity: pack
        # p's deferred tail still reads its region while pack p+1's pair-0
        # writers (skip_group_check matmuls bypass hazard tracking) run.
        #   [0:64)    tpe (bf16 view, even slots), by parity [0:32)/[32:64)
        #   [64:128)  tpc, by parity [64:96)/[96:128)
        #   [128:136) o_ps, by parity [128:132)/[132:136)
        #   [136:200) bi12  [200:206) h1  [206:207) ci1  [207:219) h
        #   [219:220) ci    [220:221) zcol  [224:288) rows
        misc = miscp.tile([128, 512], F32, name="misc")

        # ---------------- first-needed input: unit-0 fT/gT ----------------
        ebu = {}    # (pack, t) -> [32, 2048] tile
        d_ebu = {}
        def load_unit(p_, t_):
            tl = bigp.tile([32, 2048], F32R, tag=f"ebu{p_}_{t_}", name=f"ebu_{p_}_{t_}")
            ebu[(p_, t_)] = tl
            d_ebu[(p_, t_)] = nc.sync.dma_start(tl[:], eblob[p_, 32 * t_:32 * (t_ + 1), :])
        # unit (0,0) split so the first S-matmul's operands (g half 0 +
        # f chunk 0) land early instead of behind one 256KB transfer (the ACT
        # HWDGE queue is NOT used here: its first issue hides behind the
        # activation-table load)
        tl00 = bigp.tile([32, 2048], F32R, tag="ebu0_0", name="ebu_0_0")
        ebu[(0, 0)] = tl00
        nc.sync.dma_start(tl00[0:32, 1024:1536], eblob[0, 0:32, 1024:1536])
        nc.sync.dma_start(tl00[0:32, 0:128], eblob[0, 0:32, 0:128])
        nc.sync.dma_start(tl00[0:32, 128:1024], eblob[0, 0:32, 128:1024])
        nc.sync.dma_start(tl00[0:32, 1536:2048], eblob[0, 0:32, 1536:2048])
        d_ebu[(0, 0)] = None
        load_unit(0, 1)

        # ---------------- constants ----------------
        ident = consts.tile([128, 128], F32)
        d_ident = nc.sync.dma_start(ident[:], nc.inline_tensor(ident_np, name="c_ident")[:, :])
        ecols_bf = consts.tile([128, 64], BF16)
        d_ecols = nc.sync.dma_start(ecols_bf[:], nc.inline_tensor(ecols_np.astype(ml_dtypes.bfloat16), name="c_ecolsbf")[:, :])
        sel4_bf = consts.tile([128, 4], BF16)
        d_sel4 = nc.sync.dma_start(sel4_bf[:], nc.inline_tensor(sel4_np.astype(ml_dtypes.bfloat16), name="c_sel4bf")[:, :])
        selT = consts.tile([NU, BPC], F32)
        d_sel = nc.sync.dma_start(selT[:], nc.inline_tensor(sel_np, name="c_sel")[:, :])
        # tiny warm-up exp: pulls the ACT table load to t=0, overlapping input DMAs
        warm = consts.tile([1, 1], F32)
        nc.gpsimd.memset(warm[:], 0.0)
        nc.scalar.activation(warm[:], warm[:], AF.Exp)
        fc1T_sb = consts.tile([64, 64], F32)
        d_fc1 = nc.sync.dma_start(fc1T_sb[:], fc1T[:, :])
        fc1b_sb = consts.tile([64, 1], F32)
        d_fc1b = nc.sync.dma_start(fc1b_sb[:], fc1b[:, :])
        fc2T_sb = consts.tile([64, 1], F32)
        d_fc2 = nc.sync.dma_start(fc2T_sb[:], fc2T[:, :])
        ksel_np = np.zeros((3, NU), np.float32)
        for r in range(NU):
            ksel_np[r % 3, r] = 1.0
        ksel3 = consts.tile([3, NU], F32)
        d_ksel = nc.sync.dma_start(ksel3[:], nc.inline_tensor(ksel_np, name="c_ksel")[:, :])

        # ---------------- resident inputs ----------------
        nb = []
        d_nb = []
        for p in range(NPACK):
            for t_ in range(4):
                if (p, t_) not in ebu:
                    load_unit(p, t_)
            t_n = bigp.tile([128, 2048], BF16, tag="nb", bufs=2, name=f"nb_{p}")
            d_nb.append(nc.sync.dma_start(t_n[:], nblob[p, :, :]))
            nb.append(t_n)

        def guard(eng, deps):
            deps = [d for d in deps if d is not None]
            if not deps:
                return None
            n = eng.nop(nofuse=True)
            for d in deps:
                add_dep_helper(n.ins, d.ins, sync=True, reason="wait-carrier")
            return n

        def pin(inst, g):
            if g is not None:
                add_dep_helper(inst.ins, g.ins, sync=False, reason="order")

        biT_sb = tailp.tile([64, NU], F32)

        # ---------------- main loop: one unit at a time, full-unit E ----------------
        e_readers = {}   # unit -> last reader insts of its E tile
        packdata = {}
        pending_tails = []
        zl_holder = []
        eb_holder = []
        s_rot = [0]      # rotating S-buffer counter
        for rep, p in [(r_, p_) for r_ in range(repeat) for p_ in range(NPACK)]:
            csrows = packp.tile([8, 512], F32, tag="csrows", name=f"csrows_{rep}_{p}")
            # E row-0 columns land here via per-unit tiny transposes at chunk 0.
            # bf16 written to even column slots only (PSUM writes must be
            # 4-byte aligned); readers use a stride-2 view.
            tpe_ps = misc[:, 32 * (p % 2):32 * (p % 2) + 32].bitcast(BF16)
            cs_all = csps.tile([8, 512], F32, tag="cs", name=f"cs_{rep}_{p}")
            packdata[(rep, p)] = (tpe_ps, csrows, [], cs_all)
            for pair in range(2):
                tpair = (2 * pair, 2 * pair + 1)
                ud = {}
                for t in tpair:
                    u = 4 * (p + NPACK * rep) + t
                    E = epool.tile([128, NCH * 1024], BF16, tag=f"E{u % 3}", name=f"E_{u}")
                    rs = packp.tile([128, NCH], F32, tag=f"rs{t}", name=f"rs_{u}")
                    ge = guard(nc.scalar, e_readers.get(u - 3, []))
                    gev = guard(nc.vector, e_readers.get(u - 3, []))
                    ud[t] = (u, E, rs, [], gev, ge)
                # chunk-major over the pair; colsum matmuls trail by one chunk
                escr = packp.tile([128, 1024], BF16, tag=f"escr{pair}",
                                  name=f"escr_{rep}_{p}_{pair}")
                for c in range(NCH):
                    for t in tpair:
                        u, E, rs, cs_mms, g0, ge = ud[t]
                        # 3 shared S buffers: PE can write chunk c+1 while the
                        # exp of chunk c still reads its buffer (no per-unit
                        # serial S->exp->S chain)
                        S_ps = sps.tile([128, 1024], F32,
                                        tag=f"S{s_rot[0] % 3}", name=f"S_{u}_{c}")
                        s_rot[0] += 1
                        rb = 32 * t
                        eslc = ebu[(p, t)]
                        fch = eslc[0:32, 128 * c:128 * (c + 1)]
                        for h in range(2):
                            gh = eslc[0:32, 1024 + 512 * h:1024 + 512 * (h + 1)]
                            out_h = S_ps[:, 512 * h:512 * (h + 1)]
                            mm = nc.tensor.matmul(out_h, fch, gh, start=True, stop=True)
                        ech = E[:, 1024 * c:1024 * (c + 1)]
                        dve_set = DVE_CHUNKS_BY_PARITY[t % 2]
                        if c in dve_set:
                            # DVE fast-exp: int16(x*A+B) written into the bf16 E
                            # tile via bitcast; rowsum needs a separate 4x pass
                            cvt = nc.vector.tensor_scalar(
                                ech.bitcast(I16), S_ps[:], SCHR_A, SCHR_B,
                                mybir.AluOpType.mult, mybir.AluOpType.add)
                            if c == min(dve_set):
                                pin(cvt, g0)
                            nc.vector.tensor_scalar(
                                escr[:], ech, 1.0, None,
                                mybir.AluOpType.mult, mybir.AluOpType.add,
                                accum_out=rs[:, c:c + 1])
                        else:
                            # ACT exp with fused rowsum (accumulator output)
                            act = nc.scalar.activation(
                                ech, S_ps[:], AF.Exp, accum_out=rs[:, c:c + 1])
                            if c == (1 if 0 in dve_set else 0):
                                pin(act, ge)
                        if c == 2:
                            # w1-numerator columns: 8 near-free PE transposes of
                            # E row 0 ([1,128] j-slices, chunk-0 region) into
                            # tpe_ps col 4*ck+t. Emitted at chunk 2 so the PE
                            # stream never waits on chunk 0's exp.
                            tpe_l = packdata[(rep, p)][0]
                            for ck in range(NCH):
                                col = 2 * (4 * ck + t)
                                nc.tensor.transpose(
                                    tpe_l[:, col:col + 1],
                                    E[0:1, 128 * ck:128 * (ck + 1)],
                                    sel4_bf[0:1, 0:1])
                    # deferred previous-pack tail: emitted mid-pair (c==2) so
                    # its ACT/DVE work doesn't delay the first exps of the new
                    # pair (which would cascade into S-buffer reuse stalls)
                    if pair == 0 and c == 2 and pending_tails:
                        pending_tails.pop(0)()
                    # colsum matmuls for chunk c-1 (keeps PE ahead of ScalarE)
                    for t in tpair:
                        if c == 0:
                            continue
                        u, E, rs, cs_mms, g0, ge = ud[t]
                        cc = c - 1
                        for h in range(2):
                            m = nc.tensor.matmul(
                                cs_all[0:8, 0:512],
                                ecols_bf[:, 8 * (2 * t + h):8 * (2 * t + h) + 8],
                                E[:, 1024 * cc + 512 * h:1024 * cc + 512 * (h + 1)],
                                start=(t == 0 and cc == 0 and h == 0), stop=False,
                                skip_group_check=True,
                            )
                            cs_mms.append(m)
                # pair tail: final colsum chunk + w2 + o2 per unit
                for t in tpair:
                    u, E, rs, cs_mms, g0, ge = ud[t]
                    for h in range(2):
                        m = nc.tensor.matmul(
                            cs_all[0:8, 0:512],
                            ecols_bf[:, 8 * (2 * t + h):8 * (2 * t + h) + 8],
                            E[:, 1024 * (NCH - 1) + 512 * h:1024 * (NCH - 1) + 512 * (h + 1)],
                            start=False, stop=(t == 3 and h == 1),
                            skip_group_check=True,
                        )
                        cs_mms.append(m)
                    rsr = packp.tile([128, NCH], F32, tag=f"rsr{t}", name=f"rsr_{u}")
                    nc.vector.reciprocal(rsr[:], rs[:])
                    w2 = packp.tile([128, NCH], BF16, tag=f"w2{t}", name=f"w2_{u}")
                    wm = nc.vector.tensor_mul(w2[:], E[:, 0:NCH * 1024:1024], rsr[:])
                    e_readers[u] = [cs_mms[-1], wm]
                    o_ps = packdata[(rep, p)][2]
                    if t == 0:
                        o_ps.append(misc[0:64, 128 + 4 * (p % 2):132 + 4 * (p % 2)])
                    for c in range(NCH):
                        nc.tensor.matmul(
                            o_ps[0][32:64, t:t + 1],
                            nb[p][:, 512 * t + 32 * c:512 * t + 32 * (c + 1)],
                            w2[:, c:c + 1],
                            start=(c == 0), stop=(c == NCH - 1),
                            tile_position=(0, 32), skip_group_check=True,
                        )
            def make_tail(rep=rep, p=p, last=False):
                # ---- pack tail: w1 columns from tpe_ps (filled at chunk 0),
                # o1 matmuls, Bi rows
                tpe_ps, csrows, o_ps_l, cs_all = packdata[(rep, p)]
                o_ps = o_ps_l[0]
                crec = packp.tile([128, 4 * NCH], F32, tag="crec", name=f"crec_{rep}_{p}")
                ucols = packp.tile([128, 4 * NCH], BF16, tag="ucols", name=f"ucols_{rep}_{p}")
                # colsum snapshot on DVE (idle at pack starts; an ACT copy
                # here delays the next exps and cascades into S-tag stalls)
                nc.vector.tensor_copy(csrows[0:8, :], cs_all[0:8, :])
                # all 8 chunk transposes land in ONE psum tile -> one reciprocal
                tpc_ps = misc[:, 64 + 32 * (p % 2):96 + 32 * (p % 2)]
                for ck in range(NCH):
                    h8 = ck // 4
                    nc.tensor.transpose(tpc_ps[:, 4 * ck:4 * (ck + 1)],
                                        csrows[0:8, 128 * (ck % 4):128 * (ck % 4 + 1)],
                                        ident[0:8, 4 * h8:4 * h8 + 4])
                nc.vector.reciprocal(crec[:], tpc_ps[:])
                nc.vector.tensor_mul(ucols[:], tpe_ps[:, 0:2 * 4 * NCH:2], crec[:])
                for t in range(4):
                    for c in range(NCH):
                        nc.tensor.matmul(
                            o_ps[0:32, t:t + 1],
                            nb[p][:, 512 * t + 256 + 32 * c:512 * t + 256 + 32 * (c + 1)],
                            ucols[:, 4 * c + t:4 * c + t + 1],
                            start=(c == 0), stop=(c == NCH - 1),
                            skip_group_check=True,
                        )
                # o_ps IS Bi^T (f0/g0 factors are folded into nblob host-side)
                nc.vector.tensor_copy(biT_sb[:, 4 * p:4 * (p + 1)], o_ps[0:64, 0:4])
            pending_tails.append(make_tail)

        for _t in pending_tails:
            _t(last=True)
        pending_tails.clear()

        # ---------------- tail: FC + batch softmax + broadcast write ----------------
        gt1 = guard(nc.tensor, [d_fc1, d_fc1b, d_fc2, d_sel])
        h_ps = misc[0:64, 207:219]
        mmh = nc.tensor.matmul(h_ps[:], fc1T_sb[:], biT_sb[:], start=True, stop=True)
        pin(mmh, gt1)
        hT = tailp.tile([64, NU], F32)
        nc.scalar.activation(hT[:], h_ps[:], AF.Tanh, bias=fc1b_sb[:, 0:1])
        ci_ps = misc[0:NU, 219:220]
        nc.tensor.matmul(ci_ps[:], hT[:], fc2T_sb[:], start=True, stop=True)
        eci = tailp.tile([NU, 1], F32)
        nc.scalar.activation(eci[:], ci_ps[:], AF.Exp)
        if _DEBUG:
            nc.sync.dma_start(dbg_eci[:, :], eci[:])

        # single AllGather of all 12 exp-logits (the old early/late split no
        # longer helps: a first collective would occupy the CC cores until
        # after this one's input is ready anyway)
        cc_in = dramp.tile([NU, 1], F32, name="cc_in")
        cc_out = dramp.tile([NCORES * NU, 1], F32, name="cc_out")
        nc.sync.dma_start(cc_in[:], eci[:, :])
        nc.gpsimd.collective_compute(
            "AllGather",
            mybir.AluOpType.bypass,
            replica_groups=[list(range(NCORES))],
            ins=[cc_in.opt()],
            outs=[cc_out.opt()],
        )
        # collective-independent precompute (must not queue behind zl ops)
        bi12_ps = misc[0:NU, 136:200]
        nc.tensor.transpose(bi12_ps[:], biT_sb[:], ident[0:64, 0:64])
        bi12 = tailp.tile([NU, 64], F32)
        nc.vector.tensor_copy(bi12[:], bi12_ps[:])
        if _DEBUG:
            nc.sync.dma_start(dbg_bi[:, :], bi12[:])
        eciP = tailp.tile([NU, 64], BF16)
        nc.vector.tensor_scalar_mul(eciP[:], bi12[:], eci[:, 0:1])

        # collective-dependent: Z sums, alpha, rows, broadcast write
        # zl[k, (c,m)] = exp-logit of (core c, unit 3m+k): row k sums to Z_k
        zl = tailp.tile([3, 32], F32, name="zl")
        nc.sync.dma_start(zl[:, :], bass.AP(cc_out[:].tensor, 0, [[1, 3], [NU, NCORES], [3, 4]]))
        zk = tailp.tile([3, 1], F32, name="zk")
        nc.vector.reduce_sum(zk[:], zl[:], axis=mybir.AxisListType.X)
        # zcol[r] = Z_{r%3} via constant selection matmul
        zcol_ps = misc[0:NU, 220:221]
        gks = guard(nc.tensor, [d_ksel])
        mmz = nc.tensor.matmul(zcol_ps[:], ksel3[:], zk[:], start=True, stop=True)
        pin(mmz, gks)
        zr = tailp.tile([NU, 1], F32)
        nc.vector.reciprocal(zr[:], zcol_ps[:])
        # selZ = selT * (1/Z) (per-partition scalar); rows = selZ^T @ (eci*Bi)
        # selA in bf16: stationary dtype sets matmul cycles/row (1 vs 4)
        selA = tailp.tile([NU, BPC], BF16)
        nc.vector.tensor_scalar_mul(selA[:], selT[:], zr[:, 0:1])
        rows_ps = misc[0:BPC, 224:288]
        nc.tensor.matmul(rows_ps[:], selA[:], eciP[:], start=True, stop=True)
        # rep built in two free-dim halves (ACT + DVE in parallel); each
        # half's broadcast write is issued as soon as that half is ready
        rep = tailp.tile([BPC, 512], BF16)
        outr = out[:, :, :].rearrange("b (p r) d -> b p (r d)", p=128)
        nc.scalar.activation(
            rep[:, 0:256].rearrange("p (r d) -> p r d", r=4),
            rows_ps[:, None, :].broadcast_to([BPC, 4, 64]),
            AF.Copy,
        )
        nc.sync.dma_start(
            outr[:, :, 0:256],
            rep[:, None, 0:256].broadcast_to([BPC, 128, 256]),
        )
        nc.vector.tensor_copy(
            rep[:, 256:512].rearrange("p (r d) -> p r d", r=4),
            rows_ps[:, None, :].broadcast_to([BPC, 4, 64]),
        )
        nc.sync.dma_start(
            outr[:, :, 256:512],
            rep[:, None, 256:512].broadcast_to([BPC, 128, 256]),
        )
        ctx.close()
    nc.finalize()
    return nc


def make_in_maps(a_emb, v_emb, l_emb, fc1_w, fc1_b, fc2_w):
    embs = [a_emb, v_emb, l_emb]
    fc1T = np.ascontiguousarray(fc1_w.T, np.float32)           # [in, out]
    fc1b = np.ascontiguousarray(fc1_b[:, None], np.float32)    # [64, 1]
    fc2T = np.ascontiguousarray(fc2_w.T, np.float32)           # [64, 1]
    in_maps = []
    for core in range(NCORES):
        eblob = np.zeros((NPACK, 128, 2048), np.float32)
        nblob = np.zeros((NPACK, 128, 2048), ml_dtypes.bfloat16)
        for u in range(NU):
            p, t = u // 4, u % 4
            b = BPC * core + u // 3
            fi, gi = PAIRS[u % 3]
            f = embs[fi][b]  # [1024, 32]
            g = embs[gi][b]
            fT32, gT32 = f.T.astype(np.float32), g.T.astype(np.float32)
            rb = 32 * t
            eblob[p, rb:rb + 32, 0:1024] = fT32
            eblob[p, rb:rb + 32, 1024:2048] = gT32
            # natural chunked [128, 256] (chunk c at cols 32c..32c+32), with the
            # opposite tensor's row 0 folded in so the o1/o2 matmuls emit Bi^T
            fP = (f * g[0][None, :]).astype(np.float32)
            gP = (g * f[0][None, :]).astype(np.float32)
            fN = fP.reshape(NCH, 128, D).transpose(1, 0, 2).reshape(128, NCH * D)
            gN = gP.reshape(NCH, 128, D).transpose(1, 0, 2).reshape(128, NCH * D)
            nblob[p, :, 512 * t:512 * t + 256] = fN
            nblob[p, :, 512 * t + 256:512 * t + 512] = gN
        in_maps.append({
            "eblob": eblob, "nblob": nblob,
            "fc1T": fc1T, "fc1b": fc1b, "fc2T": fc2T,
        })
    return in_maps


_PROGRAM_CACHE = {}


def _get_program(repeat=1):
    key = ("nc", repeat)
    if key not in _PROGRAM_CACHE:
        _PROGRAM_CACHE[key] = build_program(repeat)
    return _PROGRAM_CACHE[key]


def kernel(a_emb, v_emb, l_emb, fc1_w, fc1_b, fc2_w, _want_results=False):
    a_emb = np.asarray(a_emb, np.float32)
    v_emb = np.asarray(v_emb, np.float32)
    l_emb = np.asarray(l_emb, np.float32)
    fc1_w = np.asarray(fc1_w, np.float32)
    fc1_b = np.asarray(fc1_b, np.float32)
    fc2_w = np.asarray(fc2_w, np.float32)
    nc = _get_program()
    in_maps = make_in_maps(a_emb, v_emb, l_emb, fc1_w, fc1_b, fc2_w)
    res = None
    for attempt in range(3):
        try:
            res = run_bass_kernel_spmd(nc, in_maps, core_ids=list(range(NCORES)))
            break
        except Exception:
            if attempt == 2:
                raise
    assert res is not None
    outp = np.concatenate(
        [np.asarray(res.results[c]["out"], np.float32) for c in range(NCORES)], axis=0)
    if _want_results:
        return outp, res
    return outp

